# revision 21
# baseline (speedup 1.0000x reference)
"""ArcFace (AngularPenaltySMLoss) on 8 TRN2 NeuronCores.

Reference math (N=4096, C=32000, S=64, M=0.5, EPS=1e-5):
    t[i]   = x[i, labels[i]]
    c[i]   = clip(t[i], -1+EPS, 1-EPS)
    num[i] = S*cos(acos(c[i]) + M) = S*(c*cos(M) - sqrt(1-c^2)*sin(M))
    den[i] = exp(num[i]) + sum_j exp(S*x[i,j]) - exp(S*t[i])
    L[i]   = num[i] - log(den[i])
    out    = -mean(L)

Sharding: data-parallel over N. Each core gets 512 rows (4 blocks of 128
partitions) and streams its [512, 32000] f32 shard from HBM once. Per
column-tile [128, W]:
  - ACT: exp(S*x) with accum_out -> per-partition partial exp-sum
  - DVE: (iota == label - tile_off) * x with accum_out -> per-partition
    partial true-logit (exactly one tile contains the label's column)
Epilogue computes L per row on-device and reduces to a [128,1] partial sum
per core; the host sums 8x128 values and scales by -1/N.
"""

import os
import sys

import numpy as np

if "/opt/trn_rl_repo" not in sys.path:
    sys.path.insert(0, "/opt/trn_rl_repo")

import concourse.bass as bass
import concourse.tile as tile
from concourse import bacc, mybir
from concourse.bass_utils import run_bass_kernel_spmd

N, C = 4096, 32000
NCORES = 8
NROW = N // NCORES          # 512 rows per core
P = 128                     # partitions
NBLK = NROW // P            # 4 row-blocks per core
W = int(os.environ.get("K_W", "4000"))       # class-tile width
NT = C // W                 # tiles per block
XBUFS = int(os.environ.get("K_XBUFS", "4"))
EBUFS = int(os.environ.get("K_EBUFS", "2"))
MBUFS = int(os.environ.get("K_MBUFS", "2"))
DMA_ALT = int(os.environ.get("K_DMA_ALT", "0"))  # alternate sync/scalar rings
DEVIOTA = int(os.environ.get("K_DEVIOTA", "0"))  # generate iota on device
S = 64.0
MARGIN = 0.5
EPS = 1e-5

F32 = mybir.dt.float32
AF = mybir.ActivationFunctionType
OP = mybir.AluOpType

LAST_RESULTS = None  # test.py reads exec_time_ns from here


def _ensure_ntff_hook():
    """The agent image's antenv lacks axon_hooks; synthesize it so
    trace=True can reach the ctypes NTFF profiling path."""
    try:
        from antenv.axon_hooks import get_axon_ntff_profile_hook  # noqa: F401

        return
    except ImportError:
        pass
    try:
        import types

        import antenv
        from trn_agent_boot.trn_boot import _ntff_profile_via_ctypes

        so_path = os.environ.get("PJRT_LIBRARY_PATH", "/opt/axon/libaxon_pjrt.so")
        hook = _ntff_profile_via_ctypes(so_path)
        mod = types.ModuleType("antenv.axon_hooks")
        mod._hook = hook
        mod.get_axon_ntff_profile_hook = lambda: mod._hook

        def _set(h):
            mod._hook = h

        mod.set_axon_ntff_profile_hook = _set
        sys.modules["antenv.axon_hooks"] = mod
        antenv.axon_hooks = mod
    except Exception as e:  # degrade to no tracing
        print(f"ntff hook install failed: {e}", file=sys.stderr)


def _build():
    import math

    nc = bacc.Bacc("TRN2", target_bir_lowering=False, debug=False)
    x_ext = nc.declare_dram_parameter("x", [NROW, C], F32, isOutput=False)
    # consts = [iota | adj] (or just adj when iota is device-generated):
    # one DMA -> one semaphore lane for all const deps
    iota_w = 0 if DEVIOTA else W
    consts_ext = nc.declare_dram_parameter(
        "consts", [P, iota_w + NBLK * NT], F32, isOutput=False
    )
    out_ext = nc.declare_dram_parameter("out", [P, 1], F32, isOutput=True)

    with tile.TileContext(nc) as tc:
        with (
            tc.tile_pool(name="xin", bufs=XBUFS) as xpool,
            tc.tile_pool(name="escr", bufs=EBUFS) as spool,
            tc.tile_pool(name="mscr", bufs=MBUFS) as mpool,
            tc.tile_pool(name="consts", bufs=1) as cpool,
            tc.tile_pool(name="acc", bufs=1) as apool,
            tc.tile_pool(name="epi", bufs=1) as epool,
        ):
            consts_t = cpool.tile([P, iota_w + NBLK * NT], F32)
            nc.sync.dma_start(consts_t[:], consts_ext[:])
            if DEVIOTA:
                iota_tile = cpool.tile([P, W], F32)
                nc.gpsimd.iota(
                    iota_tile[:], pattern=[[1, W]], base=0,
                    channel_multiplier=0,
                    allow_small_or_imprecise_dtypes=True,
                )
                iota_t = iota_tile[:]
            else:
                iota_t = consts_t[:, 0:W]

            es_acc = apool.tile([P, NBLK * NT], F32)  # partial exp-sums
            tl_acc = apool.tile([P, NBLK * NT], F32)  # partial true-logits

            # Prime DVE with a read of consts (and device iota) so per-tile
            # STT instructions only wait on their own x-tile DMA (HW
            # wait-slot limit).
            prime = cpool.tile([P, 1], F32)
            nc.vector.tensor_copy(prime[:], consts_t[:, 0:1])
            if DEVIOTA:
                nc.vector.tensor_copy(prime[:], iota_t[:, 0:1])

            for b in range(NBLK):
                for t in range(NT):
                    col = b * NT + t
                    xt = xpool.tile([P, W], F32)
                    dma_eng = nc.scalar if (DMA_ALT and col % 2) else nc.sync
                    dma_eng.dma_start(
                        xt[:], x_ext[b * P : (b + 1) * P, t * W : (t + 1) * W]
                    )
                    et = spool.tile([P, W], F32)
                    nc.scalar.activation(
                        et[:], xt[:], AF.Exp, scale=S,
                        accum_out=es_acc[:, col : col + 1],
                    )
                    mt = mpool.tile([P, W], F32)
                    nc.vector.scalar_tensor_tensor(
                        out=mt[:],
                        in0=iota_t,
                        scalar=consts_t[:, iota_w + col : iota_w + col + 1],
                        in1=xt[:],
                        op0=OP.is_equal,
                        op1=OP.mult,
                        accum_out=tl_acc[:, col : col + 1],
                    )

            # ---- epilogue: per-row loss on [128, NBLK] ----
            es = epool.tile([P, NBLK], F32)
            tl = epool.tile([P, NBLK], F32)
            for b in range(NBLK):
                nc.vector.tensor_reduce(
                    es[:, b : b + 1], es_acc[:, b * NT : (b + 1) * NT],
                    axis=mybir.AxisListType.X, op=OP.add,
                )
                nc.vector.tensor_reduce(
                    tl[:, b : b + 1], tl_acc[:, b * NT : (b + 1) * NT],
                    axis=mybir.AxisListType.X, op=OP.add,
                )

            cc = epool.tile([P, NBLK], F32)
            nc.vector.tensor_scalar(
                cc[:], tl[:], -1.0 + EPS, 1.0 - EPS, op0=OP.max, op1=OP.min
            )
            csq = epool.tile([P, NBLK], F32)
            nc.vector.tensor_mul(csq[:], cc[:], cc[:])
            om = epool.tile([P, NBLK], F32)  # 1 - c^2
            nc.vector.tensor_scalar(om[:], csq[:], -1.0, 1.0, op0=OP.mult, op1=OP.add)
            # sqrt(1-c^2) on DVE (avoids an ACT Sqrt table switch):
            # fast-inverse-sqrt seed + 2 Newton iterations, sn = om * y
            omu = om[:].bitcast(mybir.dt.int32)
            hsh = epool.tile([P, NBLK], mybir.dt.int32)
            nc.vector.tensor_scalar(
                hsh[:], omu, 1, None, op0=OP.logical_shift_right
            )
            y0u = epool.tile([P, NBLK], mybir.dt.int32)
            nc.vector.tensor_scalar(
                y0u[:], hsh[:], -1, 0x5F3759DF, op0=OP.mult, op1=OP.add
            )
            yy = epool.tile([P, NBLK], F32)
            nc.vector.tensor_copy(yy[:], y0u[:].bitcast(F32))
            nt1 = epool.tile([P, NBLK], F32)
            nt2 = epool.tile([P, NBLK], F32)
            for _ in range(2):
                nc.vector.tensor_mul(nt1[:], om[:], yy[:])
                nc.vector.tensor_mul(nt1[:], nt1[:], yy[:])
                nc.vector.tensor_scalar(
                    nt2[:], nt1[:], -0.5, 1.5, op0=OP.mult, op1=OP.add
                )
                nc.vector.tensor_mul(yy[:], yy[:], nt2[:])
            sn = epool.tile([P, NBLK], F32)  # sqrt(1-c^2)
            nc.vector.tensor_mul(sn[:], om[:], yy[:])
            ca = epool.tile([P, NBLK], F32)  # c * S*cos(M)
            nc.vector.tensor_scalar_mul(ca[:], cc[:], S * math.cos(MARGIN))
            num = epool.tile([P, NBLK], F32)  # numerator
            nc.vector.scalar_tensor_tensor(
                out=num[:], in0=sn[:], scalar=-S * math.sin(MARGIN), in1=ca[:],
                op0=OP.mult, op1=OP.add,
            )
            enum_ = epool.tile([P, NBLK], F32)
            nc.scalar.activation(enum_[:], num[:], AF.Exp)
            etl = epool.tile([P, NBLK], F32)  # exp(S * t)
            nc.scalar.activation(etl[:], tl[:], AF.Exp, scale=S)
            d1 = epool.tile([P, NBLK], F32)
            nc.vector.tensor_sub(d1[:], es[:], etl[:])
            den = epool.tile([P, NBLK], F32)
            nc.vector.tensor_add(den[:], d1[:], enum_[:])
            # ln(den) via range reduction (ACT Ln is wrong for huge inputs):
            # den = m * 2^k with m in [1,2): ln(den) = Ln(m) + k*ln2
            den_u = den[:].bitcast(mybir.dt.uint32)
            eb = epool.tile([P, NBLK], mybir.dt.uint32)
            nc.vector.tensor_scalar(
                eb[:], den_u, 23, None, op0=OP.logical_shift_right
            )
            ebf = epool.tile([P, NBLK], F32)
            nc.vector.tensor_copy(ebf[:], eb[:])
            mu = epool.tile([P, NBLK], mybir.dt.uint32)
            nc.vector.tensor_scalar(
                mu[:], den_u, 0x007FFFFF, 0x3F800000,
                op0=OP.bitwise_and, op1=OP.bitwise_or,
            )
            # ln(m), m in [1,2), on DVE (avoids an ACT Ln table switch):
            # s = (m-1)/(m+1); ln m = 2s + 2s^3/3 + 2s^5/5 + 2s^7/7
            mf = mu[:].bitcast(F32)
            mp1 = epool.tile([P, NBLK], F32)
            nc.vector.tensor_scalar(mp1[:], mf, 1.0, None, op0=OP.add)
            rcp = epool.tile([P, NBLK], F32)
            nc.vector.reciprocal(rcp[:], mp1[:])
            mm1 = epool.tile([P, NBLK], F32)
            nc.vector.tensor_scalar(mm1[:], mf, -1.0, None, op0=OP.add)
            ss = epool.tile([P, NBLK], F32)
            nc.vector.tensor_mul(ss[:], mm1[:], rcp[:])
            s2 = epool.tile([P, NBLK], F32)
            nc.vector.tensor_mul(s2[:], ss[:], ss[:])
            gg = epool.tile([P, NBLK], F32)
            nc.vector.tensor_scalar(
                gg[:], s2[:], 2.0 / 7.0, 2.0 / 5.0, op0=OP.mult, op1=OP.add
            )
            nc.vector.tensor_mul(gg[:], gg[:], s2[:])
            nc.vector.tensor_scalar(
                gg[:], gg[:], 1.0, 2.0 / 3.0, op0=OP.mult, op1=OP.add
            )
            nc.vector.tensor_mul(gg[:], gg[:], s2[:])
            nc.vector.tensor_scalar(gg[:], gg[:], 1.0, 2.0, op0=OP.mult, op1=OP.add)
            lnm = epool.tile([P, NBLK], F32)
            nc.vector.tensor_mul(lnm[:], gg[:], ss[:])
            kln2 = epool.tile([P, NBLK], F32)
            ln2 = math.log(2.0)
            nc.vector.tensor_scalar(
                kln2[:], ebf[:], ln2, -127.0 * ln2, op0=OP.mult, op1=OP.add
            )
            ln = epool.tile([P, NBLK], F32)
            nc.vector.tensor_add(ln[:], lnm[:], kln2[:])
            ll = epool.tile([P, NBLK], F32)
            nc.vector.tensor_sub(ll[:], num[:], ln[:])
            lred = epool.tile([P, 1], F32)
            nc.vector.tensor_reduce(
                lred[:], ll[:], axis=mybir.AxisListType.X, op=OP.add
            )
            nc.sync.dma_start(out_ext[:], lred[:])

    nc.finalize()
    return nc


def kernel(x: np.ndarray, labels: np.ndarray) -> np.ndarray:
    global LAST_RESULTS
    x = np.ascontiguousarray(np.asarray(x, dtype=np.float32))
    labels = np.asarray(labels).astype(np.int64)
    assert x.shape == (N, C) and labels.shape == (N,)

    iota = np.broadcast_to(np.arange(W, dtype=np.float32)[None, :], (P, W))

    in_maps = []
    for k in range(NCORES):
        xs = np.ascontiguousarray(x[k * NROW : (k + 1) * NROW])
        ls = labels[k * NROW : (k + 1) * NROW].reshape(NBLK, P)
        # adj[p, b*NT + t] = label[b*P + p] - t*W
        adj = (
            ls.T[:, :, None].astype(np.float32)
            - (np.arange(NT, dtype=np.float32) * W)[None, None, :]
        ).reshape(P, NBLK * NT)
        parts = ([adj] if DEVIOTA else [iota, adj])
        consts = np.ascontiguousarray(
            np.concatenate(parts, axis=1), dtype=np.float32
        )
        in_maps.append({"x": xs, "consts": consts})

    nc = _build()
    trace = bool(os.environ.get("BASS_TRACE"))
    if trace:
        _ensure_ntff_hook()
    res = run_bass_kernel_spmd(
        nc, in_maps, core_ids=list(range(NCORES)), trace=trace
    )
    LAST_RESULTS = res
    total = np.float64(0.0)
    for r in res.results:
        total += np.sum(np.asarray(r["out"], dtype=np.float64))
    return np.float32(-total / N)


# revision 22
# speedup vs baseline: 1.0312x; 1.0312x over previous
"""ArcFace (AngularPenaltySMLoss) on 8 TRN2 NeuronCores.

Reference math (N=4096, C=32000, S=64, M=0.5, EPS=1e-5):
    t[i]   = x[i, labels[i]]
    c[i]   = clip(t[i], -1+EPS, 1-EPS)
    num[i] = S*cos(acos(c[i]) + M) = S*(c*cos(M) - sqrt(1-c^2)*sin(M))
    den[i] = exp(num[i]) + sum_j exp(S*x[i,j]) - exp(S*t[i])
    L[i]   = num[i] - log(den[i])
    out    = -mean(L)

Sharding: data-parallel over N. Each core gets 512 rows (4 blocks of 128
partitions) and streams its [512, 32000] f32 shard from HBM once. Per
column-tile [128, W]:
  - ACT: exp(S*x) with accum_out -> per-partition partial exp-sum
  - DVE: (iota == label - tile_off) * x with accum_out -> per-partition
    partial true-logit (exactly one tile contains the label's column)
Epilogue computes L per row on-device and reduces to a [128,1] partial sum
per core; the host sums 8x128 values and scales by -1/N.
"""

import os
import sys

import numpy as np

if "/opt/trn_rl_repo" not in sys.path:
    sys.path.insert(0, "/opt/trn_rl_repo")

import concourse.bass as bass
import concourse.tile as tile
from concourse import bacc, mybir
from concourse.bass_utils import run_bass_kernel_spmd

N, C = 4096, 32000
NCORES = 8
NROW = N // NCORES          # 512 rows per core
P = 128                     # partitions
NBLK = NROW // P            # 4 row-blocks per core
W = int(os.environ.get("K_W", "4000"))       # class-tile width
NT = C // W                 # tiles per block
XBUFS = int(os.environ.get("K_XBUFS", "6"))
EBUFS = int(os.environ.get("K_EBUFS", "2"))
MBUFS = int(os.environ.get("K_MBUFS", "2"))
DMA_ALT = int(os.environ.get("K_DMA_ALT", "0"))  # alternate sync/scalar rings
DEVIOTA = int(os.environ.get("K_DEVIOTA", "1"))  # generate iota on device
S = 64.0
MARGIN = 0.5
EPS = 1e-5

F32 = mybir.dt.float32
AF = mybir.ActivationFunctionType
OP = mybir.AluOpType

LAST_RESULTS = None  # test.py reads exec_time_ns from here


def _ensure_ntff_hook():
    """The agent image's antenv lacks axon_hooks; synthesize it so
    trace=True can reach the ctypes NTFF profiling path."""
    try:
        from antenv.axon_hooks import get_axon_ntff_profile_hook  # noqa: F401

        return
    except ImportError:
        pass
    try:
        import types

        import antenv
        from trn_agent_boot.trn_boot import _ntff_profile_via_ctypes

        so_path = os.environ.get("PJRT_LIBRARY_PATH", "/opt/axon/libaxon_pjrt.so")
        hook = _ntff_profile_via_ctypes(so_path)
        mod = types.ModuleType("antenv.axon_hooks")
        mod._hook = hook
        mod.get_axon_ntff_profile_hook = lambda: mod._hook

        def _set(h):
            mod._hook = h

        mod.set_axon_ntff_profile_hook = _set
        sys.modules["antenv.axon_hooks"] = mod
        antenv.axon_hooks = mod
    except Exception as e:  # degrade to no tracing
        print(f"ntff hook install failed: {e}", file=sys.stderr)


def _build():
    import math

    nc = bacc.Bacc("TRN2", target_bir_lowering=False, debug=False)
    x_ext = nc.declare_dram_parameter("x", [NROW, C], F32, isOutput=False)
    # consts = [iota | adj] (or just adj when iota is device-generated):
    # one DMA -> one semaphore lane for all const deps
    iota_w = 0 if DEVIOTA else W
    consts_ext = nc.declare_dram_parameter(
        "consts", [P, iota_w + NBLK * NT], F32, isOutput=False
    )
    out_ext = nc.declare_dram_parameter("out", [P, 1], F32, isOutput=True)

    with tile.TileContext(nc) as tc:
        with (
            tc.tile_pool(name="xin", bufs=XBUFS) as xpool,
            tc.tile_pool(name="escr", bufs=EBUFS) as spool,
            tc.tile_pool(name="mscr", bufs=MBUFS) as mpool,
            tc.tile_pool(name="consts", bufs=1) as cpool,
            tc.tile_pool(name="acc", bufs=1) as apool,
            tc.tile_pool(name="epi", bufs=1) as epool,
        ):
            consts_t = cpool.tile([P, iota_w + NBLK * NT], F32)
            nc.sync.dma_start(consts_t[:], consts_ext[:])
            if DEVIOTA:
                iota_tile = cpool.tile([P, W], F32)
                nc.gpsimd.iota(
                    iota_tile[:], pattern=[[1, W]], base=0,
                    channel_multiplier=0,
                    allow_small_or_imprecise_dtypes=True,
                )
                iota_t = iota_tile[:]
            else:
                iota_t = consts_t[:, 0:W]

            es_acc = apool.tile([P, NBLK * NT], F32)  # partial exp-sums
            tl_acc = apool.tile([P, NBLK * NT], F32)  # partial true-logits

            # Prime DVE with a read of consts (and device iota) so per-tile
            # STT instructions only wait on their own x-tile DMA (HW
            # wait-slot limit).
            prime = cpool.tile([P, 1], F32)
            nc.vector.tensor_copy(prime[:], consts_t[:, 0:1])
            if DEVIOTA:
                nc.vector.tensor_copy(prime[:], iota_t[:, 0:1])

            for b in range(NBLK):
                for t in range(NT):
                    col = b * NT + t
                    xt = xpool.tile([P, W], F32)
                    dma_eng = nc.scalar if (DMA_ALT and col % 2) else nc.sync
                    dma_eng.dma_start(
                        xt[:], x_ext[b * P : (b + 1) * P, t * W : (t + 1) * W]
                    )
                    et = spool.tile([P, W], F32)
                    nc.scalar.activation(
                        et[:], xt[:], AF.Exp, scale=S,
                        accum_out=es_acc[:, col : col + 1],
                    )
                    mt = mpool.tile([P, W], F32)
                    nc.vector.scalar_tensor_tensor(
                        out=mt[:],
                        in0=iota_t,
                        scalar=consts_t[:, iota_w + col : iota_w + col + 1],
                        in1=xt[:],
                        op0=OP.is_equal,
                        op1=OP.mult,
                        accum_out=tl_acc[:, col : col + 1],
                    )

            # ---- epilogue: per-row loss on [128, NBLK] ----
            es = epool.tile([P, NBLK], F32)
            tl = epool.tile([P, NBLK], F32)
            for b in range(NBLK):
                nc.vector.tensor_reduce(
                    es[:, b : b + 1], es_acc[:, b * NT : (b + 1) * NT],
                    axis=mybir.AxisListType.X, op=OP.add,
                )
                nc.vector.tensor_reduce(
                    tl[:, b : b + 1], tl_acc[:, b * NT : (b + 1) * NT],
                    axis=mybir.AxisListType.X, op=OP.add,
                )

            cc = epool.tile([P, NBLK], F32)
            nc.vector.tensor_scalar(
                cc[:], tl[:], -1.0 + EPS, 1.0 - EPS, op0=OP.max, op1=OP.min
            )
            csq = epool.tile([P, NBLK], F32)
            nc.vector.tensor_mul(csq[:], cc[:], cc[:])
            om = epool.tile([P, NBLK], F32)  # 1 - c^2
            nc.vector.tensor_scalar(om[:], csq[:], -1.0, 1.0, op0=OP.mult, op1=OP.add)
            # sqrt(1-c^2) on DVE (avoids an ACT Sqrt table switch):
            # fast-inverse-sqrt seed + 2 Newton iterations, sn = om * y
            omu = om[:].bitcast(mybir.dt.int32)
            hsh = epool.tile([P, NBLK], mybir.dt.int32)
            nc.vector.tensor_scalar(
                hsh[:], omu, 1, None, op0=OP.logical_shift_right
            )
            y0u = epool.tile([P, NBLK], mybir.dt.int32)
            nc.vector.tensor_scalar(
                y0u[:], hsh[:], -1, 0x5F3759DF, op0=OP.mult, op1=OP.add
            )
            yy = epool.tile([P, NBLK], F32)
            nc.vector.tensor_copy(yy[:], y0u[:].bitcast(F32))
            nt1 = epool.tile([P, NBLK], F32)
            nt2 = epool.tile([P, NBLK], F32)
            for _ in range(2):
                nc.vector.tensor_mul(nt1[:], om[:], yy[:])
                nc.vector.tensor_mul(nt1[:], nt1[:], yy[:])
                nc.vector.tensor_scalar(
                    nt2[:], nt1[:], -0.5, 1.5, op0=OP.mult, op1=OP.add
                )
                nc.vector.tensor_mul(yy[:], yy[:], nt2[:])
            sn = epool.tile([P, NBLK], F32)  # sqrt(1-c^2)
            nc.vector.tensor_mul(sn[:], om[:], yy[:])
            ca = epool.tile([P, NBLK], F32)  # c * S*cos(M)
            nc.vector.tensor_scalar_mul(ca[:], cc[:], S * math.cos(MARGIN))
            num = epool.tile([P, NBLK], F32)  # numerator
            nc.vector.scalar_tensor_tensor(
                out=num[:], in0=sn[:], scalar=-S * math.sin(MARGIN), in1=ca[:],
                op0=OP.mult, op1=OP.add,
            )
            enum_ = epool.tile([P, NBLK], F32)
            nc.scalar.activation(enum_[:], num[:], AF.Exp)
            etl = epool.tile([P, NBLK], F32)  # exp(S * t)
            nc.scalar.activation(etl[:], tl[:], AF.Exp, scale=S)
            d1 = epool.tile([P, NBLK], F32)
            nc.vector.tensor_sub(d1[:], es[:], etl[:])
            den = epool.tile([P, NBLK], F32)
            nc.vector.tensor_add(den[:], d1[:], enum_[:])
            # ln(den) via range reduction (ACT Ln is wrong for huge inputs):
            # den = m * 2^k with m in [1,2): ln(den) = Ln(m) + k*ln2
            den_u = den[:].bitcast(mybir.dt.uint32)
            eb = epool.tile([P, NBLK], mybir.dt.uint32)
            nc.vector.tensor_scalar(
                eb[:], den_u, 23, None, op0=OP.logical_shift_right
            )
            ebf = epool.tile([P, NBLK], F32)
            nc.vector.tensor_copy(ebf[:], eb[:])
            mu = epool.tile([P, NBLK], mybir.dt.uint32)
            nc.vector.tensor_scalar(
                mu[:], den_u, 0x007FFFFF, 0x3F800000,
                op0=OP.bitwise_and, op1=OP.bitwise_or,
            )
            # ln(m), m in [1,2), on DVE (avoids an ACT Ln table switch):
            # s = (m-1)/(m+1); ln m = 2s + 2s^3/3 + 2s^5/5 + 2s^7/7
            mf = mu[:].bitcast(F32)
            mp1 = epool.tile([P, NBLK], F32)
            nc.vector.tensor_scalar(mp1[:], mf, 1.0, None, op0=OP.add)
            rcp = epool.tile([P, NBLK], F32)
            nc.vector.reciprocal(rcp[:], mp1[:])
            mm1 = epool.tile([P, NBLK], F32)
            nc.vector.tensor_scalar(mm1[:], mf, -1.0, None, op0=OP.add)
            ss = epool.tile([P, NBLK], F32)
            nc.vector.tensor_mul(ss[:], mm1[:], rcp[:])
            s2 = epool.tile([P, NBLK], F32)
            nc.vector.tensor_mul(s2[:], ss[:], ss[:])
            gg = epool.tile([P, NBLK], F32)
            nc.vector.tensor_scalar(
                gg[:], s2[:], 2.0 / 7.0, 2.0 / 5.0, op0=OP.mult, op1=OP.add
            )
            nc.vector.tensor_mul(gg[:], gg[:], s2[:])
            nc.vector.tensor_scalar(
                gg[:], gg[:], 1.0, 2.0 / 3.0, op0=OP.mult, op1=OP.add
            )
            nc.vector.tensor_mul(gg[:], gg[:], s2[:])
            nc.vector.tensor_scalar(gg[:], gg[:], 1.0, 2.0, op0=OP.mult, op1=OP.add)
            lnm = epool.tile([P, NBLK], F32)
            nc.vector.tensor_mul(lnm[:], gg[:], ss[:])
            kln2 = epool.tile([P, NBLK], F32)
            ln2 = math.log(2.0)
            nc.vector.tensor_scalar(
                kln2[:], ebf[:], ln2, -127.0 * ln2, op0=OP.mult, op1=OP.add
            )
            ln = epool.tile([P, NBLK], F32)
            nc.vector.tensor_add(ln[:], lnm[:], kln2[:])
            ll = epool.tile([P, NBLK], F32)
            nc.vector.tensor_sub(ll[:], num[:], ln[:])
            lred = epool.tile([P, 1], F32)
            nc.vector.tensor_reduce(
                lred[:], ll[:], axis=mybir.AxisListType.X, op=OP.add
            )
            nc.sync.dma_start(out_ext[:], lred[:])

    nc.finalize()
    return nc


def kernel(x: np.ndarray, labels: np.ndarray) -> np.ndarray:
    global LAST_RESULTS
    x = np.ascontiguousarray(np.asarray(x, dtype=np.float32))
    labels = np.asarray(labels).astype(np.int64)
    assert x.shape == (N, C) and labels.shape == (N,)

    iota = np.broadcast_to(np.arange(W, dtype=np.float32)[None, :], (P, W))

    in_maps = []
    for k in range(NCORES):
        xs = np.ascontiguousarray(x[k * NROW : (k + 1) * NROW])
        ls = labels[k * NROW : (k + 1) * NROW].reshape(NBLK, P)
        # adj[p, b*NT + t] = label[b*P + p] - t*W
        adj = (
            ls.T[:, :, None].astype(np.float32)
            - (np.arange(NT, dtype=np.float32) * W)[None, None, :]
        ).reshape(P, NBLK * NT)
        parts = ([adj] if DEVIOTA else [iota, adj])
        consts = np.ascontiguousarray(
            np.concatenate(parts, axis=1), dtype=np.float32
        )
        in_maps.append({"x": xs, "consts": consts})

    nc = _build()
    trace = bool(os.environ.get("BASS_TRACE"))
    if trace:
        _ensure_ntff_hook()
    res = run_bass_kernel_spmd(
        nc, in_maps, core_ids=list(range(NCORES)), trace=trace
    )
    LAST_RESULTS = res
    total = np.float64(0.0)
    for r in res.results:
        total += np.sum(np.asarray(r["out"], dtype=np.float64))
    return np.float32(-total / N)


# revision 24
# speedup vs baseline: 1.0890x; 1.0561x over previous
"""ArcFace (AngularPenaltySMLoss) on 8 TRN2 NeuronCores.

Reference math (N=4096, C=32000, S=64, M=0.5, EPS=1e-5):
    t[i]   = x[i, labels[i]]
    c[i]   = clip(t[i], -1+EPS, 1-EPS)
    num[i] = S*cos(acos(c[i]) + M) = S*(c*cos(M) - sqrt(1-c^2)*sin(M))
    den[i] = exp(num[i]) + sum_j exp(S*x[i,j]) - exp(S*t[i])
    L[i]   = num[i] - log(den[i])
    out    = -mean(L)

Sharding: data-parallel over N. Each core gets 512 rows (4 blocks of 128
partitions) and streams its [512, 32000] f32 shard from HBM once. Per
column-tile [128, W]:
  - ACT: exp(S*x) with accum_out -> per-partition partial exp-sum
  - DVE: (iota == label - tile_off) * x with accum_out -> per-partition
    partial true-logit (exactly one tile contains the label's column)
Epilogue computes L per row on-device and reduces to a [128,1] partial sum
per core; the host sums 8x128 values and scales by -1/N.
"""

import os
import sys

import numpy as np

if "/opt/trn_rl_repo" not in sys.path:
    sys.path.insert(0, "/opt/trn_rl_repo")

import concourse.bass as bass
import concourse.tile as tile
from concourse import bacc, mybir
from concourse.bass_utils import run_bass_kernel_spmd

N, C = 4096, 32000
NCORES = 8
NROW = N // NCORES          # 512 rows per core
P = 128                     # partitions
NBLK = NROW // P            # 4 row-blocks per core
W = int(os.environ.get("K_W", "4000"))       # class-tile width
NT = C // W                 # tiles per block
XBUFS = int(os.environ.get("K_XBUFS", "6"))
EBUFS = int(os.environ.get("K_EBUFS", "2"))
MBUFS = int(os.environ.get("K_MBUFS", "2"))
DMA_ALT = int(os.environ.get("K_DMA_ALT", "0"))  # alternate sync/scalar rings
DEVIOTA = int(os.environ.get("K_DEVIOTA", "1"))  # generate iota on device
S = 64.0
MARGIN = 0.5
EPS = 1e-5

F32 = mybir.dt.float32
AF = mybir.ActivationFunctionType
OP = mybir.AluOpType

LAST_RESULTS = None  # test.py reads exec_time_ns from here

SKIP_BARRIER2 = int(os.environ.get("K_SKIP_BARRIER2", "1"))


def _patch_tail_barrier():
    """Drop the second all-engine EVSEM butterfly in Tile's kernel tail.

    Tile ends a kernel with: drain(waits on global clock) -> barrier ->
    sem_clear -> barrier. The first three are load-bearing (the clear must
    not race in-flight sem increments, and cleared sems are what make the
    NEFF re-executable since target_bir_lowering=False skips start-of-kernel
    clears). The final barrier only orders the clears against nothing —
    NEFF completion already requires every engine queue to drain — so skip
    it to shave the tail.
    """
    from concourse.vector_clock import ScopedClock

    def _drain_and_barrier(self, tick_clock, wait_clock):
        drain_inst = self.nc.sync.drain()
        wait_clock.add_sem_waits(
            drain_inst.ins, ScopedClock({None: tick_clock.global_clock})
        )
        self.nc.all_engine_barrier()
        assert self.sems is not None
        popped = self.nc._tile_sem_poison_stack.pop()
        assert popped is self._sem_poison
        self.nc.clear_and_free_semaphores(list(self.sems.allocated().values()))

    tile.TileContext._drain_and_barrier = _drain_and_barrier


def _ensure_ntff_hook():
    """The agent image's antenv lacks axon_hooks; synthesize it so
    trace=True can reach the ctypes NTFF profiling path."""
    try:
        from antenv.axon_hooks import get_axon_ntff_profile_hook  # noqa: F401

        return
    except ImportError:
        pass
    try:
        import types

        import antenv
        from trn_agent_boot.trn_boot import _ntff_profile_via_ctypes

        so_path = os.environ.get("PJRT_LIBRARY_PATH", "/opt/axon/libaxon_pjrt.so")
        hook = _ntff_profile_via_ctypes(so_path)
        mod = types.ModuleType("antenv.axon_hooks")
        mod._hook = hook
        mod.get_axon_ntff_profile_hook = lambda: mod._hook

        def _set(h):
            mod._hook = h

        mod.set_axon_ntff_profile_hook = _set
        sys.modules["antenv.axon_hooks"] = mod
        antenv.axon_hooks = mod
    except Exception as e:  # degrade to no tracing
        print(f"ntff hook install failed: {e}", file=sys.stderr)


def _build():
    import math

    if SKIP_BARRIER2:
        _patch_tail_barrier()
    nc = bacc.Bacc("TRN2", target_bir_lowering=False, debug=False)
    x_ext = nc.declare_dram_parameter("x", [NROW, C], F32, isOutput=False)
    # consts = [iota | adj] (or just adj when iota is device-generated):
    # one DMA -> one semaphore lane for all const deps
    iota_w = 0 if DEVIOTA else W
    consts_ext = nc.declare_dram_parameter(
        "consts", [P, iota_w + NBLK * NT], F32, isOutput=False
    )
    out_ext = nc.declare_dram_parameter("out", [P, 1], F32, isOutput=True)

    with tile.TileContext(nc) as tc:
        with (
            tc.tile_pool(name="xin", bufs=XBUFS) as xpool,
            tc.tile_pool(name="escr", bufs=EBUFS) as spool,
            tc.tile_pool(name="mscr", bufs=MBUFS) as mpool,
            tc.tile_pool(name="consts", bufs=1) as cpool,
            tc.tile_pool(name="acc", bufs=1) as apool,
            tc.tile_pool(name="epi", bufs=1) as epool,
        ):
            consts_t = cpool.tile([P, iota_w + NBLK * NT], F32)
            nc.sync.dma_start(consts_t[:], consts_ext[:])
            if DEVIOTA:
                iota_tile = cpool.tile([P, W], F32)
                nc.gpsimd.iota(
                    iota_tile[:], pattern=[[1, W]], base=0,
                    channel_multiplier=0,
                    allow_small_or_imprecise_dtypes=True,
                )
                iota_t = iota_tile[:]
            else:
                iota_t = consts_t[:, 0:W]

            es_acc = apool.tile([P, NBLK * NT], F32)  # partial exp-sums
            tl_acc = apool.tile([P, NBLK * NT], F32)  # partial true-logits

            # Prime DVE with a read of consts (and device iota) so per-tile
            # STT instructions only wait on their own x-tile DMA (HW
            # wait-slot limit).
            prime = cpool.tile([P, 1], F32)
            nc.vector.tensor_copy(prime[:], consts_t[:, 0:1])
            if DEVIOTA:
                nc.vector.tensor_copy(prime[:], iota_t[:, 0:1])

            for b in range(NBLK):
                for t in range(NT):
                    col = b * NT + t
                    xt = xpool.tile([P, W], F32)
                    dma_eng = nc.scalar if (DMA_ALT and col % 2) else nc.sync
                    dma_eng.dma_start(
                        xt[:], x_ext[b * P : (b + 1) * P, t * W : (t + 1) * W]
                    )
                    et = spool.tile([P, W], F32)
                    nc.scalar.activation(
                        et[:], xt[:], AF.Exp, scale=S,
                        accum_out=es_acc[:, col : col + 1],
                    )
                    mt = mpool.tile([P, W], F32)
                    nc.vector.scalar_tensor_tensor(
                        out=mt[:],
                        in0=iota_t,
                        scalar=consts_t[:, iota_w + col : iota_w + col + 1],
                        in1=xt[:],
                        op0=OP.is_equal,
                        op1=OP.mult,
                        accum_out=tl_acc[:, col : col + 1],
                    )

            # ---- epilogue: per-row loss on [128, NBLK] ----
            es = epool.tile([P, NBLK], F32)
            tl = epool.tile([P, NBLK], F32)
            for b in range(NBLK):
                nc.vector.tensor_reduce(
                    es[:, b : b + 1], es_acc[:, b * NT : (b + 1) * NT],
                    axis=mybir.AxisListType.X, op=OP.add,
                )
                nc.vector.tensor_reduce(
                    tl[:, b : b + 1], tl_acc[:, b * NT : (b + 1) * NT],
                    axis=mybir.AxisListType.X, op=OP.add,
                )

            cc = epool.tile([P, NBLK], F32)
            nc.vector.tensor_scalar(
                cc[:], tl[:], -1.0 + EPS, 1.0 - EPS, op0=OP.max, op1=OP.min
            )
            csq = epool.tile([P, NBLK], F32)
            nc.vector.tensor_mul(csq[:], cc[:], cc[:])
            om = epool.tile([P, NBLK], F32)  # 1 - c^2
            nc.vector.tensor_scalar(om[:], csq[:], -1.0, 1.0, op0=OP.mult, op1=OP.add)
            # sqrt(1-c^2) on DVE (avoids an ACT Sqrt table switch):
            # fast-inverse-sqrt seed + 2 Newton iterations, sn = om * y
            omu = om[:].bitcast(mybir.dt.int32)
            hsh = epool.tile([P, NBLK], mybir.dt.int32)
            nc.vector.tensor_scalar(
                hsh[:], omu, 1, None, op0=OP.logical_shift_right
            )
            y0u = epool.tile([P, NBLK], mybir.dt.int32)
            nc.vector.tensor_scalar(
                y0u[:], hsh[:], -1, 0x5F3759DF, op0=OP.mult, op1=OP.add
            )
            yy = epool.tile([P, NBLK], F32)
            nc.vector.tensor_copy(yy[:], y0u[:].bitcast(F32))
            nt1 = epool.tile([P, NBLK], F32)
            nt2 = epool.tile([P, NBLK], F32)
            for _ in range(2):
                nc.vector.tensor_mul(nt1[:], om[:], yy[:])
                nc.vector.tensor_mul(nt1[:], nt1[:], yy[:])
                nc.vector.tensor_scalar(
                    nt2[:], nt1[:], -0.5, 1.5, op0=OP.mult, op1=OP.add
                )
                nc.vector.tensor_mul(yy[:], yy[:], nt2[:])
            sn = epool.tile([P, NBLK], F32)  # sqrt(1-c^2)
            nc.vector.tensor_mul(sn[:], om[:], yy[:])
            ca = epool.tile([P, NBLK], F32)  # c * S*cos(M)
            nc.vector.tensor_scalar_mul(ca[:], cc[:], S * math.cos(MARGIN))
            num = epool.tile([P, NBLK], F32)  # numerator
            nc.vector.scalar_tensor_tensor(
                out=num[:], in0=sn[:], scalar=-S * math.sin(MARGIN), in1=ca[:],
                op0=OP.mult, op1=OP.add,
            )
            enum_ = epool.tile([P, NBLK], F32)
            nc.scalar.activation(enum_[:], num[:], AF.Exp)
            etl = epool.tile([P, NBLK], F32)  # exp(S * t)
            nc.scalar.activation(etl[:], tl[:], AF.Exp, scale=S)
            d1 = epool.tile([P, NBLK], F32)
            nc.vector.tensor_sub(d1[:], es[:], etl[:])
            den = epool.tile([P, NBLK], F32)
            nc.vector.tensor_add(den[:], d1[:], enum_[:])
            # ln(den) via range reduction (ACT Ln is wrong for huge inputs):
            # den = m * 2^k with m in [1,2): ln(den) = Ln(m) + k*ln2
            den_u = den[:].bitcast(mybir.dt.uint32)
            eb = epool.tile([P, NBLK], mybir.dt.uint32)
            nc.vector.tensor_scalar(
                eb[:], den_u, 23, None, op0=OP.logical_shift_right
            )
            ebf = epool.tile([P, NBLK], F32)
            nc.vector.tensor_copy(ebf[:], eb[:])
            mu = epool.tile([P, NBLK], mybir.dt.uint32)
            nc.vector.tensor_scalar(
                mu[:], den_u, 0x007FFFFF, 0x3F800000,
                op0=OP.bitwise_and, op1=OP.bitwise_or,
            )
            # ln(m), m in [1,2), on DVE (avoids an ACT Ln table switch):
            # s = (m-1)/(m+1); ln m = 2s + 2s^3/3 + 2s^5/5 + 2s^7/7
            mf = mu[:].bitcast(F32)
            mp1 = epool.tile([P, NBLK], F32)
            nc.vector.tensor_scalar(mp1[:], mf, 1.0, None, op0=OP.add)
            rcp = epool.tile([P, NBLK], F32)
            nc.vector.reciprocal(rcp[:], mp1[:])
            mm1 = epool.tile([P, NBLK], F32)
            nc.vector.tensor_scalar(mm1[:], mf, -1.0, None, op0=OP.add)
            ss = epool.tile([P, NBLK], F32)
            nc.vector.tensor_mul(ss[:], mm1[:], rcp[:])
            s2 = epool.tile([P, NBLK], F32)
            nc.vector.tensor_mul(s2[:], ss[:], ss[:])
            gg = epool.tile([P, NBLK], F32)
            nc.vector.tensor_scalar(
                gg[:], s2[:], 2.0 / 7.0, 2.0 / 5.0, op0=OP.mult, op1=OP.add
            )
            nc.vector.tensor_mul(gg[:], gg[:], s2[:])
            nc.vector.tensor_scalar(
                gg[:], gg[:], 1.0, 2.0 / 3.0, op0=OP.mult, op1=OP.add
            )
            nc.vector.tensor_mul(gg[:], gg[:], s2[:])
            nc.vector.tensor_scalar(gg[:], gg[:], 1.0, 2.0, op0=OP.mult, op1=OP.add)
            lnm = epool.tile([P, NBLK], F32)
            nc.vector.tensor_mul(lnm[:], gg[:], ss[:])
            kln2 = epool.tile([P, NBLK], F32)
            ln2 = math.log(2.0)
            nc.vector.tensor_scalar(
                kln2[:], ebf[:], ln2, -127.0 * ln2, op0=OP.mult, op1=OP.add
            )
            ln = epool.tile([P, NBLK], F32)
            nc.vector.tensor_add(ln[:], lnm[:], kln2[:])
            ll = epool.tile([P, NBLK], F32)
            nc.vector.tensor_sub(ll[:], num[:], ln[:])
            lred = epool.tile([P, 1], F32)
            nc.vector.tensor_reduce(
                lred[:], ll[:], axis=mybir.AxisListType.X, op=OP.add
            )
            nc.sync.dma_start(out_ext[:], lred[:])

    nc.finalize()
    return nc


def kernel(x: np.ndarray, labels: np.ndarray) -> np.ndarray:
    global LAST_RESULTS
    x = np.ascontiguousarray(np.asarray(x, dtype=np.float32))
    labels = np.asarray(labels).astype(np.int64)
    assert x.shape == (N, C) and labels.shape == (N,)

    iota = np.broadcast_to(np.arange(W, dtype=np.float32)[None, :], (P, W))

    in_maps = []
    for k in range(NCORES):
        xs = np.ascontiguousarray(x[k * NROW : (k + 1) * NROW])
        ls = labels[k * NROW : (k + 1) * NROW].reshape(NBLK, P)
        # adj[p, b*NT + t] = label[b*P + p] - t*W
        adj = (
            ls.T[:, :, None].astype(np.float32)
            - (np.arange(NT, dtype=np.float32) * W)[None, None, :]
        ).reshape(P, NBLK * NT)
        parts = ([adj] if DEVIOTA else [iota, adj])
        consts = np.ascontiguousarray(
            np.concatenate(parts, axis=1), dtype=np.float32
        )
        in_maps.append({"x": xs, "consts": consts})

    nc = _build()
    trace = bool(os.environ.get("BASS_TRACE"))
    if trace:
        _ensure_ntff_hook()
    res = run_bass_kernel_spmd(
        nc, in_maps, core_ids=list(range(NCORES)), trace=trace
    )
    LAST_RESULTS = res
    total = np.float64(0.0)
    for r in res.results:
        total += np.sum(np.asarray(r["out"], dtype=np.float64))
    return np.float32(-total / N)


# revision 26
# speedup vs baseline: 1.2612x; 1.1580x over previous
"""ArcFace (AngularPenaltySMLoss) on 8 TRN2 NeuronCores.

Reference math (N=4096, C=32000, S=64, M=0.5, EPS=1e-5):
    t[i]   = x[i, labels[i]]
    c[i]   = clip(t[i], -1+EPS, 1-EPS)
    num[i] = S*cos(acos(c[i]) + M) = S*(c*cos(M) - sqrt(1-c^2)*sin(M))
    den[i] = exp(num[i]) + sum_j exp(S*x[i,j]) - exp(S*t[i])
    L[i]   = num[i] - log(den[i])
    out    = -mean(L)

Sharding: data-parallel over N. Each core gets 512 rows (4 blocks of 128
partitions) and streams its [512, 32000] f32 shard from HBM once. Per
column-tile [128, W]:
  - ACT: exp(S*x) with accum_out -> per-partition partial exp-sum
  - DVE: (iota == label - tile_off) * x with accum_out -> per-partition
    partial true-logit (exactly one tile contains the label's column)
Epilogue computes L per row on-device and reduces to a [128,1] partial sum
per core; the host sums 8x128 values and scales by -1/N.
"""

import os
import sys

import numpy as np

if "/opt/trn_rl_repo" not in sys.path:
    sys.path.insert(0, "/opt/trn_rl_repo")

import concourse.bass as bass
import concourse.tile as tile
from concourse import bacc, mybir
from concourse.bass_utils import run_bass_kernel_spmd

N, C = 4096, 32000
NCORES = 8
NROW = N // NCORES          # 512 rows per core
P = 128                     # partitions
NBLK = NROW // P            # 4 row-blocks per core
W = int(os.environ.get("K_W", "4000"))       # class-tile width
NT = C // W                 # tiles per block
XBUFS = int(os.environ.get("K_XBUFS", "6"))
EBUFS = int(os.environ.get("K_EBUFS", "2"))
MBUFS = int(os.environ.get("K_MBUFS", "2"))
DMA_ALT = int(os.environ.get("K_DMA_ALT", "0"))  # alternate sync/scalar rings
DEVIOTA = int(os.environ.get("K_DEVIOTA", "1"))  # generate iota on device
S = 64.0
MARGIN = 0.5
EPS = 1e-5

F32 = mybir.dt.float32
AF = mybir.ActivationFunctionType
OP = mybir.AluOpType

LAST_RESULTS = None  # test.py reads exec_time_ns from here

SKIP_BARRIER2 = int(os.environ.get("K_SKIP_BARRIER2", "1"))


def _patch_tail_barrier():
    """Drop the second all-engine EVSEM butterfly in Tile's kernel tail.

    Tile ends a kernel with: drain(waits on global clock) -> barrier ->
    sem_clear -> barrier. The first three are load-bearing (the clear must
    not race in-flight sem increments, and cleared sems are what make the
    NEFF re-executable since target_bir_lowering=False skips start-of-kernel
    clears). The final barrier only orders the clears against nothing —
    NEFF completion already requires every engine queue to drain — so skip
    it to shave the tail.
    """
    from concourse.vector_clock import ScopedClock

    def _drain_and_barrier(self, tick_clock, wait_clock):
        drain_inst = self.nc.sync.drain()
        wait_clock.add_sem_waits(
            drain_inst.ins, ScopedClock({None: tick_clock.global_clock})
        )
        self.nc.all_engine_barrier()
        assert self.sems is not None
        popped = self.nc._tile_sem_poison_stack.pop()
        assert popped is self._sem_poison
        # Skip gpsimd.dma_reset inside clear_and_free: NRT expands the
        # ranged drain into ~176 per-DMA-queue ops (~8us on silicon). The
        # global-clock drain above already guarantees every DMA sem
        # increment has landed, so only the sem_clear is needed for
        # re-execution.
        gps_cls = type(self.nc.gpsimd)
        orig_reset = gps_cls.dma_reset
        gps_cls.dma_reset = lambda _self, semaphore_range=None: None
        try:
            self.nc.clear_and_free_semaphores(
                list(self.sems.allocated().values())
            )
        finally:
            gps_cls.dma_reset = orig_reset

    tile.TileContext._drain_and_barrier = _drain_and_barrier


def _ensure_ntff_hook():
    """The agent image's antenv lacks axon_hooks; synthesize it so
    trace=True can reach the ctypes NTFF profiling path."""
    try:
        from antenv.axon_hooks import get_axon_ntff_profile_hook  # noqa: F401

        return
    except ImportError:
        pass
    try:
        import types

        import antenv
        from trn_agent_boot.trn_boot import _ntff_profile_via_ctypes

        so_path = os.environ.get("PJRT_LIBRARY_PATH", "/opt/axon/libaxon_pjrt.so")
        hook = _ntff_profile_via_ctypes(so_path)
        mod = types.ModuleType("antenv.axon_hooks")
        mod._hook = hook
        mod.get_axon_ntff_profile_hook = lambda: mod._hook

        def _set(h):
            mod._hook = h

        mod.set_axon_ntff_profile_hook = _set
        sys.modules["antenv.axon_hooks"] = mod
        antenv.axon_hooks = mod
    except Exception as e:  # degrade to no tracing
        print(f"ntff hook install failed: {e}", file=sys.stderr)


def _build():
    import math

    if SKIP_BARRIER2:
        _patch_tail_barrier()
    nc = bacc.Bacc("TRN2", target_bir_lowering=False, debug=False)
    x_ext = nc.declare_dram_parameter("x", [NROW, C], F32, isOutput=False)
    # consts = [iota | adj] (or just adj when iota is device-generated):
    # one DMA -> one semaphore lane for all const deps
    iota_w = 0 if DEVIOTA else W
    consts_ext = nc.declare_dram_parameter(
        "consts", [P, iota_w + NBLK * NT], F32, isOutput=False
    )
    out_ext = nc.declare_dram_parameter("out", [P, 1], F32, isOutput=True)

    with tile.TileContext(nc) as tc:
        with (
            tc.tile_pool(name="xin", bufs=XBUFS) as xpool,
            tc.tile_pool(name="escr", bufs=EBUFS) as spool,
            tc.tile_pool(name="mscr", bufs=MBUFS) as mpool,
            tc.tile_pool(name="consts", bufs=1) as cpool,
            tc.tile_pool(name="acc", bufs=1) as apool,
            tc.tile_pool(name="epi", bufs=1) as epool,
        ):
            consts_t = cpool.tile([P, iota_w + NBLK * NT], F32)
            nc.sync.dma_start(consts_t[:], consts_ext[:])
            if DEVIOTA:
                iota_tile = cpool.tile([P, W], F32)
                nc.gpsimd.iota(
                    iota_tile[:], pattern=[[1, W]], base=0,
                    channel_multiplier=0,
                    allow_small_or_imprecise_dtypes=True,
                )
                iota_t = iota_tile[:]
            else:
                iota_t = consts_t[:, 0:W]

            es_acc = apool.tile([P, NBLK * NT], F32)  # partial exp-sums
            tl_acc = apool.tile([P, NBLK * NT], F32)  # partial true-logits

            # Prime DVE with a read of consts (and device iota) so per-tile
            # STT instructions only wait on their own x-tile DMA (HW
            # wait-slot limit).
            prime = cpool.tile([P, 1], F32)
            nc.vector.tensor_copy(prime[:], consts_t[:, 0:1])
            if DEVIOTA:
                nc.vector.tensor_copy(prime[:], iota_t[:, 0:1])

            for b in range(NBLK):
                for t in range(NT):
                    col = b * NT + t
                    xt = xpool.tile([P, W], F32)
                    dma_eng = nc.scalar if (DMA_ALT and col % 2) else nc.sync
                    dma_eng.dma_start(
                        xt[:], x_ext[b * P : (b + 1) * P, t * W : (t + 1) * W]
                    )
                    et = spool.tile([P, W], F32)
                    nc.scalar.activation(
                        et[:], xt[:], AF.Exp, scale=S,
                        accum_out=es_acc[:, col : col + 1],
                    )
                    mt = mpool.tile([P, W], F32)
                    nc.vector.scalar_tensor_tensor(
                        out=mt[:],
                        in0=iota_t,
                        scalar=consts_t[:, iota_w + col : iota_w + col + 1],
                        in1=xt[:],
                        op0=OP.is_equal,
                        op1=OP.mult,
                        accum_out=tl_acc[:, col : col + 1],
                    )

            # ---- epilogue: per-row loss on [128, NBLK] ----
            es = epool.tile([P, NBLK], F32)
            tl = epool.tile([P, NBLK], F32)
            for b in range(NBLK):
                nc.vector.tensor_reduce(
                    es[:, b : b + 1], es_acc[:, b * NT : (b + 1) * NT],
                    axis=mybir.AxisListType.X, op=OP.add,
                )
                nc.vector.tensor_reduce(
                    tl[:, b : b + 1], tl_acc[:, b * NT : (b + 1) * NT],
                    axis=mybir.AxisListType.X, op=OP.add,
                )

            cc = epool.tile([P, NBLK], F32)
            nc.vector.tensor_scalar(
                cc[:], tl[:], -1.0 + EPS, 1.0 - EPS, op0=OP.max, op1=OP.min
            )
            csq = epool.tile([P, NBLK], F32)
            nc.vector.tensor_mul(csq[:], cc[:], cc[:])
            om = epool.tile([P, NBLK], F32)  # 1 - c^2
            nc.vector.tensor_scalar(om[:], csq[:], -1.0, 1.0, op0=OP.mult, op1=OP.add)
            # sqrt(1-c^2) on DVE (avoids an ACT Sqrt table switch):
            # fast-inverse-sqrt seed + 2 Newton iterations, sn = om * y
            omu = om[:].bitcast(mybir.dt.int32)
            hsh = epool.tile([P, NBLK], mybir.dt.int32)
            nc.vector.tensor_scalar(
                hsh[:], omu, 1, None, op0=OP.logical_shift_right
            )
            y0u = epool.tile([P, NBLK], mybir.dt.int32)
            nc.vector.tensor_scalar(
                y0u[:], hsh[:], -1, 0x5F3759DF, op0=OP.mult, op1=OP.add
            )
            yy = epool.tile([P, NBLK], F32)
            nc.vector.tensor_copy(yy[:], y0u[:].bitcast(F32))
            nt1 = epool.tile([P, NBLK], F32)
            nt2 = epool.tile([P, NBLK], F32)
            for _ in range(2):  # 2 iters: rel err ~4e-6 (1 iter ~1.2e-3)
                nc.vector.tensor_mul(nt1[:], om[:], yy[:])
                nc.vector.tensor_mul(nt1[:], nt1[:], yy[:])
                nc.vector.tensor_scalar(
                    nt2[:], nt1[:], -0.5, 1.5, op0=OP.mult, op1=OP.add
                )
                nc.vector.tensor_mul(yy[:], yy[:], nt2[:])
            sn = epool.tile([P, NBLK], F32)  # sqrt(1-c^2)
            nc.vector.tensor_mul(sn[:], om[:], yy[:])
            ca = epool.tile([P, NBLK], F32)  # c * S*cos(M)
            nc.vector.tensor_scalar_mul(ca[:], cc[:], S * math.cos(MARGIN))
            num = epool.tile([P, NBLK], F32)  # numerator
            nc.vector.scalar_tensor_tensor(
                out=num[:], in0=sn[:], scalar=-S * math.sin(MARGIN), in1=ca[:],
                op0=OP.mult, op1=OP.add,
            )
            enum_ = epool.tile([P, NBLK], F32)
            nc.scalar.activation(enum_[:], num[:], AF.Exp)
            etl = epool.tile([P, NBLK], F32)  # exp(S * t)
            nc.scalar.activation(etl[:], tl[:], AF.Exp, scale=S)
            d1 = epool.tile([P, NBLK], F32)
            nc.vector.tensor_sub(d1[:], es[:], etl[:])
            den = epool.tile([P, NBLK], F32)
            nc.vector.tensor_add(den[:], d1[:], enum_[:])
            # ln(den) via range reduction (ACT Ln is wrong for huge inputs):
            # den = m * 2^k with m in [1,2): ln(den) = Ln(m) + k*ln2
            den_u = den[:].bitcast(mybir.dt.uint32)
            eb = epool.tile([P, NBLK], mybir.dt.uint32)
            nc.vector.tensor_scalar(
                eb[:], den_u, 23, None, op0=OP.logical_shift_right
            )
            ebf = epool.tile([P, NBLK], F32)
            nc.vector.tensor_copy(ebf[:], eb[:])
            mu = epool.tile([P, NBLK], mybir.dt.uint32)
            nc.vector.tensor_scalar(
                mu[:], den_u, 0x007FFFFF, 0x3F800000,
                op0=OP.bitwise_and, op1=OP.bitwise_or,
            )
            # ln(m), m in [1,2), on DVE (avoids an ACT Ln table switch):
            # s = (m-1)/(m+1); ln m = 2s + 2s^3/3 + 2s^5/5 + 2s^7/7
            mf = mu[:].bitcast(F32)
            mp1 = epool.tile([P, NBLK], F32)
            nc.vector.tensor_scalar(mp1[:], mf, 1.0, None, op0=OP.add)
            rcp = epool.tile([P, NBLK], F32)
            nc.vector.reciprocal(rcp[:], mp1[:])
            mm1 = epool.tile([P, NBLK], F32)
            nc.vector.tensor_scalar(mm1[:], mf, -1.0, None, op0=OP.add)
            ss = epool.tile([P, NBLK], F32)
            nc.vector.tensor_mul(ss[:], mm1[:], rcp[:])
            s2 = epool.tile([P, NBLK], F32)
            nc.vector.tensor_mul(s2[:], ss[:], ss[:])
            gg = epool.tile([P, NBLK], F32)
            nc.vector.tensor_scalar(
                gg[:], s2[:], 2.0 / 7.0, 2.0 / 5.0, op0=OP.mult, op1=OP.add
            )
            nc.vector.tensor_mul(gg[:], gg[:], s2[:])
            nc.vector.tensor_scalar(
                gg[:], gg[:], 1.0, 2.0 / 3.0, op0=OP.mult, op1=OP.add
            )
            nc.vector.tensor_mul(gg[:], gg[:], s2[:])
            nc.vector.tensor_scalar(gg[:], gg[:], 1.0, 2.0, op0=OP.mult, op1=OP.add)
            lnm = epool.tile([P, NBLK], F32)
            nc.vector.tensor_mul(lnm[:], gg[:], ss[:])
            kln2 = epool.tile([P, NBLK], F32)
            ln2 = math.log(2.0)
            nc.vector.tensor_scalar(
                kln2[:], ebf[:], ln2, -127.0 * ln2, op0=OP.mult, op1=OP.add
            )
            ln = epool.tile([P, NBLK], F32)
            nc.vector.tensor_add(ln[:], lnm[:], kln2[:])
            ll = epool.tile([P, NBLK], F32)
            nc.vector.tensor_sub(ll[:], num[:], ln[:])
            lred = epool.tile([P, 1], F32)
            nc.vector.tensor_reduce(
                lred[:], ll[:], axis=mybir.AxisListType.X, op=OP.add
            )
            nc.sync.dma_start(out_ext[:], lred[:])

    nc.finalize()
    return nc


def kernel(x: np.ndarray, labels: np.ndarray) -> np.ndarray:
    global LAST_RESULTS
    x = np.ascontiguousarray(np.asarray(x, dtype=np.float32))
    labels = np.asarray(labels).astype(np.int64)
    assert x.shape == (N, C) and labels.shape == (N,)

    iota = np.broadcast_to(np.arange(W, dtype=np.float32)[None, :], (P, W))

    in_maps = []
    for k in range(NCORES):
        xs = np.ascontiguousarray(x[k * NROW : (k + 1) * NROW])
        ls = labels[k * NROW : (k + 1) * NROW].reshape(NBLK, P)
        # adj[p, b*NT + t] = label[b*P + p] - t*W
        adj = (
            ls.T[:, :, None].astype(np.float32)
            - (np.arange(NT, dtype=np.float32) * W)[None, None, :]
        ).reshape(P, NBLK * NT)
        parts = ([adj] if DEVIOTA else [iota, adj])
        consts = np.ascontiguousarray(
            np.concatenate(parts, axis=1), dtype=np.float32
        )
        in_maps.append({"x": xs, "consts": consts})

    nc = _build()
    trace = bool(os.environ.get("BASS_TRACE"))
    if trace:
        _ensure_ntff_hook()
    res = run_bass_kernel_spmd(
        nc, in_maps, core_ids=list(range(NCORES)), trace=trace
    )
    LAST_RESULTS = res
    total = np.float64(0.0)
    for r in res.results:
        total += np.sum(np.asarray(r["out"], dtype=np.float64))
    return np.float32(-total / N)
